# revision 61
# baseline (speedup 1.0000x reference)
"""DetectionLoss Trainium2 kernel (8-core data parallel), v3.

Per-core: 64 samples; groups of 2 samples -> 128 partitions = (s, t).
Free dim = 1280 preds (10 tiles of 128, 104 pad).

Score (order-equivalent to IoU): S = dx*dy/(ab+at) with
  dx = relu(WT - relu(bx1-tx1) - relu(tx2-bx2))        (exact)
5 PE streams per chunk, K=3 weights [selA, selB, target-row] so the
target offsets ride in the matmul; one 4-stream ACT relu evacuates
i1x,i1y,i2x,i2y from a 4-bank PSUM tile; the SAB stream is negated in
its weights so the final sign works out with eym<=0.

fp16 everywhere on SBUF (DVE 2x modes); fp32 only in PSUM + reciprocal.
No GPSIMD compute in the hot loop (Pool shares the DVE SBUF port and
poisons VEC throughput); Pool only issues DMAs / partition_broadcast.

Gather of matched box/logits/conf via exact index-compare one-hot
(transposed) matmuls; pos-mask term skips multiplicity dedup (error
~1.5e-4, measured).
"""

import numpy as np
import os
from contextlib import ExitStack

import concourse.bass as bass
import concourse.mybir as mybir
from concourse import bacc, tile
from concourse.bass_utils import run_bass_kernel_spmd

F32 = mybir.dt.float32
F16 = mybir.dt.float16
I32 = mybir.dt.int32
U32 = mybir.dt.uint32
OP = mybir.AluOpType
AF = mybir.ActivationFunctionType
AX = mybir.AxisListType

B, N, T, C = 512, 1176, 64, 4
NCORES = 8
BC = B // NCORES          # samples per core = 64
NG = BC // 2              # groups of 2 samples = 32
NJ = 10                   # pred tiles of 128 (padded)
NN = NJ * 128             # 1280
NTAIL = N - 9 * 128       # 24 valid rows in tile j=9
CHUNKS = [(0, 512), (512, 512), (1024, 256)]
IMG_W, IMG_H = 1472.0, 832.0
CS = 0.25                 # coordinate scale for fp16 range
LN4 = float(np.log(np.float64(4.0)))
SQRT_HALF = float(np.sqrt(np.float64(0.5)))
NV = 9                    # V cols per sample: 4 box, 4 logits, 1 conf
DBG = os.environ.get("KDEBUG", "") == "1"


def build_kernel():
    nc = bacc.Bacc(
        "TRN2",
        target_bir_lowering=False,
        debug=False,
        enable_asserts=False,
        num_devices=NCORES,
    )
    pred_d = nc.dram_tensor("predictions", [BC, N, 9], F32, kind="ExternalInput").ap()
    tb_d = nc.dram_tensor("target_boxes", [BC, T, 4], F32, kind="ExternalInput").ap()
    tc_d = nc.dram_tensor("target_classes", [BC, T], I32, kind="ExternalInput").ap()
    csel_d = nc.dram_tensor("csel", [2, NG * 5, 128], F16,
                            kind="ExternalInput").ap()
    out_d = nc.dram_tensor("out", [4], F32, kind="ExternalOutput").ap()
    dbg_d = (nc.dram_tensor("dbg", [128, NG], F32, kind="ExternalOutput").ap()
             if DBG else None)
    dbgs_d = (nc.dram_tensor("dbgS", [NG, 128, NN], F32, kind="ExternalOutput").ap()
              if DBG else None)
    dbgg_d = (nc.dram_tensor("dbgG", [128, NG, 32], F32, kind="ExternalOutput").ap()
              if DBG else None)

    with tile.TileContext(nc) as tcx:
        with ExitStack() as ctx:
            emit(ctx, tcx, pred_d, tb_d, tc_d, csel_d, out_d, dbg_d, dbgs_d,
                 dbgg_d)
    nc.compile()
    return nc


def emit(ctx, tcx, pred_d, tb_d, tc_d, csel_d, out_d, dbg_d=None, dbgs_d=None,
         dbgg_d=None):
    nc = tcx.nc
    tp = lambda name, bufs, **kw: ctx.enter_context(
        tcx.tile_pool(name=name, bufs=bufs, **kw)
    )

    const_p = tp("const", 1)
    big_p = tp("big", 1)
    work_p = tp("work", 2)
    wide_p = tp("wide", 1)
    sc_p = tp("sc", 2)
    small_p = tp("small", 3)
    stage_p = tp("stage", 2)
    psQ_p = tp("psQ", 1, space="PSUM")
    psB_p = tp("psB", 2, space="PSUM")
    psG_p = tp("psG", 1, space="PSUM")
    rs_p = tp("rs", 2)

    vec = nc.vector
    act = nc.scalar
    gps = nc.gpsimd

    # ---------------- constants ----------------
    onescol = const_p.tile([128, 1], F32, tag="onescol")
    vec.memset(onescol[:, :], 1.0)
    ONESROW = const_p.tile([1, 128], F16, tag="ONESROW")
    vec.memset(ONESROW[:, :], 1.0)
    # PJC[p, j, col] = p + 128*j  (fp16, exact up to 2048)
    pjc_i = const_p.tile([128, NJ, 128], mybir.dt.int16, tag="pjc_i")
    gps.iota(pjc_i[:, :, :], pattern=[[128, NJ], [0, 128]], base=0,
             channel_multiplier=1)
    PJC = const_p.tile([128, NJ, 128], F16, tag="PJC")
    vec.tensor_copy(PJC[:, :, :], pjc_i[:, :, :])
    # partition half masks
    MA = const_p.tile([128, 2], F32, tag="MA")
    vec.memset(MA[:, :], 0.0)
    vec.memset(MA[0:64, 0:1], 1.0)
    vec.memset(MA[64:128, 1:2], 1.0)

    # ---------------- loads ----------------
    # X[p, s, j, k] = pred[s, j*128+p, k]; pad rows zeroed, conf col -> -100
    X = big_p.tile([128, BC, NJ, 9], F32, tag="X")
    vec.memset(X[:, :, 9, :], 0.0)
    vec.memset(X[:, :, 9, 4], -100.0)
    nsplit = 2
    sw = BC // nsplit
    for i in range(nsplit):
        s0 = i * sw
        for j in range(NJ):
            pw = 128 if j < 9 else NTAIL
            eng = [nc.sync, nc.gpsimd, nc.scalar][(i * NJ + j) % 3]
            eng.dma_start(
                X[0:pw, s0 : s0 + sw, j, :],
                pred_d[s0 : s0 + sw, j * 128 : j * 128 + pw, :].rearrange(
                    "s p k -> p s k"
                ),
            )

    # targets: TGTC[p=(s,t), g, c] fp32
    TGTC = big_p.tile([128, NG, 4], F32, tag="TGTC")
    nc.sync.dma_start(TGTC[:, :, :], tb_d.rearrange("(g s) t c -> (s t) g c", s=2))
    TCI = big_p.tile([128, NG], I32, tag="TCI")
    nc.sync.dma_start(TCI[:, :], tc_d.rearrange("(g s) t -> (s t) g", s=2))
    TCF = big_p.tile([128, NG], F32, tag="TCF")
    vec.tensor_copy(TCF[:, :], TCI[:, :])

    # scaled target scalars (fp32):
    #  TGS cols: 0 tx1*cs, 1 ty1*cs, 2 tx2*cs, 3 ty2*cs, 4 wt*cs, 5 ht*cs,
    #            6 at*cs*cs
    TGS = big_p.tile([128, NG, 7], F32, tag="TGS")
    vec.tensor_scalar(TGS[:, :, 0:4], TGTC[:, :, :], CS, None, OP.mult)
    vec.tensor_tensor(TGS[:, :, 4], TGS[:, :, 2], TGS[:, :, 0], OP.subtract)
    vec.tensor_tensor(TGS[:, :, 5], TGS[:, :, 3], TGS[:, :, 1], OP.subtract)
    vec.tensor_tensor(TGS[:, :, 6], TGS[:, :, 4], TGS[:, :, 5], OP.mult)

    # ---------------- decode (fp32 in -> fp16 out) ----------------
    WHX = big_p.tile([128, BC, NJ], F16, tag="WHX")
    WHY = big_p.tile([128, BC, NJ], F16, tag="WHY")
    CX = big_p.tile([128, BC, NJ], F16, tag="CX")
    CY = big_p.tile([128, BC, NJ], F16, tag="CY")
    cb = const_p.tile([128, 3], F32, tag="cb")
    vec.memset(cb[:, 0:1], LN4)
    vec.memset(cb[:, 1:2], -IMG_W / 2 * CS)
    vec.memset(cb[:, 2:3], -IMG_H / 2 * CS)
    act.activation(WHX[:, :, :], X[:, :, :, 2], AF.Exp, bias=cb[:, 0:1], scale=1.0)
    act.activation(WHY[:, :, :], X[:, :, :, 3], AF.Exp, bias=cb[:, 0:1], scale=1.0)
    act.activation(CX[:, :, :], X[:, :, :, 0], AF.Identity, bias=cb[:, 1:2],
                   scale=IMG_W * CS)
    act.activation(CY[:, :, :], X[:, :, :, 1], AF.Identity, bias=cb[:, 2:3],
                   scale=IMG_H * CS)

    # DQ16[p, s, q*NJ+j], q: 0 bx1s, 1 by1s, 2 nbx2s, 3 nby2s, 4 abcc
    # (free dim padded to 128 for the XBAR DMA transpose)
    DQ16 = big_p.tile([128, BC, 128], F16, tag="DQ16")
    DQv = DQ16[:, :, 0 : 5 * NJ].rearrange("p s (q j) -> p s q j", q=5)
    vec.tensor_tensor(DQv[:, :, 0, :], CX[:, :, :], WHX[:, :, :], OP.subtract)
    vec.tensor_tensor(DQv[:, :, 1, :], CY[:, :, :], WHY[:, :, :], OP.subtract)
    vec.scalar_tensor_tensor(DQv[:, :, 2, :], CX[:, :, :], -1.0, WHX[:, :, :],
                             OP.mult, OP.subtract)
    vec.scalar_tensor_tensor(DQv[:, :, 3, :], CY[:, :, :], -1.0, WHY[:, :, :],
                             OP.mult, OP.subtract)
    vec.scalar_tensor_tensor(DQv[:, :, 4, :], WHX[:, :, :], 4.0, WHY[:, :, :],
                             OP.mult, OP.mult)

    # X16V[p, s, c, j]: c 0..3 = box (bx1s, by1s, nbx2s, nby2s), 4..7 logits,
    # 8 conf. fp16 gather source (lhsT of the gather matmuls).
    X16V = big_p.tile([128, BC, NV, NJ], F16, tag="X16V")
    vec.tensor_copy(
        X16V[:, :, 0:4, :],
        DQ16[:, :, 0 : 4 * NJ].rearrange("p s (c j) -> p s c j", c=4),
    )
    act.activation(
        X16V[:, :, 4:8, :],
        X[:, :, :, 5:9].rearrange("p s j c -> p s c j"),
        AF.Copy,
    )
    vec.tensor_copy(X16V[:, :, 8, :], X[:, :, :, 4])

    # one-hot class targets for CE
    Y = big_p.tile([128, NG, C], F32, tag="Y")
    for cc in range(C):
        vec.tensor_scalar(Y[:, :, cc], TCF[:, :], float(cc), None, OP.is_equal)

    # ---------------- K=3 stream weights ----------------
    # SEL5[:, f=g*5+q, :]: rows (selA, selB, target-row(q, g)).
    # q4 rows are NEGATED (so rs<0 and S = dx*(-eym)*(-rs) >= 0 works out).
    # target rows: q0 -tx1s, q1 -ty1s, q2 +tx2s, q3 +ty2s, q4 -at*cc.
    T5cc = stage_p.tile([128, 256], F16, tag="T5cc")
    vec.memset(T5cc[:, :], 0.0)
    vec.tensor_scalar(T5cc[:, 0 : 5 * NG : 5], TGS[:, :, 0], -1.0, None, OP.mult)
    vec.tensor_scalar(T5cc[:, 1 : 5 * NG : 5], TGS[:, :, 1], -1.0, None, OP.mult)
    vec.tensor_copy(T5cc[:, 2 : 5 * NG : 5], TGS[:, :, 2])
    vec.tensor_copy(T5cc[:, 3 : 5 * NG : 5], TGS[:, :, 3])
    vec.tensor_scalar(T5cc[:, 4 : 5 * NG : 5], TGS[:, :, 6], -1.0, None, OP.mult)
    T5a = stage_p.tile([128, 128], F16, tag="T5a")
    T5b = stage_p.tile([128, 128], F16, tag="T5b")
    nc.sync.dma_start_transpose(T5a[:, :], T5cc[:, 0:128])
    nc.sync.dma_start_transpose(T5b[:, :], T5cc[:, 128:256])
    SEL5 = const_p.tile([3, NG * 5, 128], F16, tag="SEL5")
    nc.sync.dma_start(SEL5[0:2, :, :], csel_d[:, :, :])
    nc.scalar.dma_start(SEL5[2:3, 0:128, :], T5a[:, :])
    nc.scalar.dma_start(SEL5[2:3, 128 : 5 * NG, :], T5b[0 : 5 * NG - 128, :])

    # persistent pred-row tiles (manually double-buffered; row2 = ones,
    # written once here, rows 0/1 rewritten per group by the reshape DMAs)
    PRS = [big_p.tile([3, 5, NN], F16, tag=f"PR{i}", name=f"PR{i}")
           for i in range(2)]
    for pr in PRS:
        gps.dma_start(
            pr[2:3, :, :].rearrange("one q (j p) -> one (q j) p", p=128),
            ONESROW[:, :].unsqueeze(1).broadcast_to((1, 5 * NJ, 128)),
        )

    # accumulators
    ACCB = big_p.tile([128, 8], F32, tag="ACCB")
    ACCC = big_p.tile([128, 4], F32, tag="ACCC")
    vec.memset(ACCB[:, :], 0.0)
    vec.memset(ACCC[:, :], 0.0)
    GALL = big_p.tile([128, NG, 32], F16, tag="GALL")
    junk16 = big_p.tile([128, BC * NJ], F16, tag="junk16")

    # ---------------- per-group software pipeline ----------------
    # iteration g: emit PE matmuls + ACT evacs for group g, interleaved with
    # the full post chain (scores/argmax/gather) for group g-1, so every
    # engine's in-order queue always has ready work.
    def prep(g):
        PR = PRS[g % 2]
        for s in range(2):
            Tst = stage_p.tile([128, 128], F16, tag="Tst", name="Tst")
            nc.sync.dma_start_transpose(Tst[:, :], DQ16[:, 2 * g + s, :])
            gps.dma_start(
                PR[s : s + 1, :, :].rearrange("one q (j p) -> one (q j) p",
                                              p=128),
                Tst[0 : 5 * NJ, :],
            )

    def mm_stage(g):
        PR = PRS[g % 2]
        R4 = work_p.tile([128, 4, NN], F16, tag="R4", name="R4")
        psBs = []
        for ci, (c0, cw) in enumerate(CHUNKS):
            psA = psQ_p.tile([128, 4, 512], F32, tag="psA", name="psA")
            psB = psB_p.tile([128, 512], F32, tag="psB", name="psB")
            for q in range(4):
                nc.tensor.matmul(
                    psA[:, q, 0:cw], SEL5[:, g * 5 + q, :],
                    PR[:, q, c0 : c0 + cw], start=True, stop=True,
                )
            nc.tensor.matmul(
                psB[:, 0:cw], SEL5[:, g * 5 + 4, :], PR[:, 4, c0 : c0 + cw],
                start=True, stop=True,
            )
            # one ACT op evacuates all 4 coordinate streams (relu, fp16)
            act.activation(R4[:, :, c0 : c0 + cw], psA[:, :, 0:cw], AF.Relu)
            psBs.append(psB)
        return R4, psBs

    def recip_chunk(st, ci):
        c0, cw = CHUNKS[ci]
        vec.reciprocal_approx_fast(st["RS32"][:, c0 : c0 + cw],
                                   st["psBs"][ci][:, 0:cw])

    def post_stage(g, st):
        R4 = st["R4"]
        RS32 = st["RS32"]
        SXY = wide_p.tile([128, 2, NN], F16, tag="SXY", name="SXY")
        vec.tensor_tensor(SXY[:, :, :], R4[:, 0:2, :], R4[:, 2:4, :], OP.add)
        DX = wide_p.tile([128, NN], F16, tag="DX", name="DX")
        act.activation(DX[:, :], SXY[:, 0, :], AF.Relu, bias=TGS[:, g, 4:5],
                       scale=-1.0)
        DY = wide_p.tile([128, NN], F16, tag="DY", name="DY")
        vec.tensor_scalar(DY[:, :], SXY[:, 1, :], TGS[:, g, 5:6], 0.0,
                          OP.subtract, OP.min)
        IPN = wide_p.tile([128, NN], F16, tag="IPN", name="IPN")
        vec.tensor_tensor(IPN[:, :], DX[:, :], DY[:, :], OP.mult)
        RS16 = wide_p.tile([128, NN], F16, tag="RS16", name="RS16")
        act.activation(RS16[:, :], RS32[:, :], AF.Copy)
        S = sc_p.tile([128, NN], F16, tag="S", name="S")
        vec.tensor_tensor(S[:, :], IPN[:, :], RS16[:, :], OP.mult)

        v8t = small_p.tile([128, 8], F16, tag="v8t", name="v8t")
        vec.max(v8t[:, :], S[:, :])
        idx8 = small_p.tile([128, 8], U32, tag="idx8", name="idx8")
        vec.max_index(idx8[:, :], v8t[:, :], S[:, :])
        mf16 = small_p.tile([128, 1], F16, tag="mf16", name="mf16")
        vec.tensor_copy(mf16[:, :], idx8[:, 0:1])
        if DBG:
            mf32 = small_p.tile([128, 1], F32, tag="mf32", name="mf32")
            vec.tensor_copy(mf32[:, :], idx8[:, 0:1])
            nc.sync.dma_start(dbg_d[:, g : g + 1], mf32[:, :])
            S32 = sc_p.tile([128, NN], F32, tag="S32", name="S32")
            vec.tensor_copy(S32[:, :], S[:, :])
            nc.sync.dma_start(dbgs_d[g, :, :], S32[:, :])

        MROW = small_p.tile([1, 128], F16, tag="MROW", name="MROW")
        gps.dma_start(
            MROW[:, :].rearrange("one (p x) -> one p x", x=1), mf16[:, :]
        )
        MB = small_p.tile([128, 128], F16, tag="MB", name="MB")
        gps.partition_broadcast(MB[:, :], MROW[:, :])
        st["MB"] = MB

    def oht_gather(g, st):
        OHT = sc_p.tile([128, NJ, 128], F16, tag="OHT", name="OHT")
        vec.tensor_tensor(
            OHT[:, :, :], PJC[:, :, :],
            st["MB"][:, :].unsqueeze(1).broadcast_to((128, NJ, 128)),
            OP.is_equal,
        )
        GT = psG_p.tile([2 * NV, 128], F32, tag="GT", name="GT")
        for j in range(NJ):
            nc.tensor.matmul(
                GT[:, :],
                X16V[:, 2 * g : 2 * g + 2, :, j].rearrange("p s c -> p (s c)"),
                OHT[:, j, :],
                start=(j == 0), stop=(j == NJ - 1),
            )
        GTS = stage_p.tile([32, 128], F16, tag="GTS", name="GTS")
        act.activation(GTS[0 : 2 * NV, :], GT[:, :], AF.Copy)
        nc.sync.dma_start_transpose(GALL[:, g, :], GTS[:, :])
        if DBG:
            G32 = small_p.tile([128, 32], F32, tag="G32", name="G32")
            vec.tensor_copy(G32[:, :], GALL[:, g, :])
            nc.sync.dma_start(dbgg_d[:, g, :], G32[:, :])

    prep(0)
    states = {}
    for g in range(NG + 2):
        if g < NG:
            if g + 1 < NG:
                prep(g + 1)
            R4, psBs = mm_stage(g)
            RS32 = rs_p.tile([128, NN], F32, tag="RS32", name="RS32")
            states[g] = {"R4": R4, "psBs": psBs, "RS32": RS32}
            recip_chunk(states[g], 0)
        if 1 <= g <= NG:
            post_stage(g - 1, states[g - 1])
        if g < NG:
            recip_chunk(states[g], 1)
            recip_chunk(states[g], 2)
        if g >= 2:
            oht_gather(g - 2, states[g - 2])
            del states[g - 2]

    # ---------------- losses ----------------
    # box smooth-l1: d = |sgn*4*g - t|; sl1 = 0.5*min(d,1)^2 + max(d,1) - 1
    for s in range(2):
        P = slice(64 * s, 64 * s + 64)
        co = NV * s
        for pi, (csl, sgn) in enumerate([(slice(0, 2), 4.0), (slice(2, 4), -4.0)]):
            D = small_p.tile([128, NG, 2], F32, tag="D")
            AD = small_p.tile([128, NG, 2], F32, tag="AD")
            DM = small_p.tile([128, NG, 2], F16, tag="DM")
            vec.scalar_tensor_tensor(
                D[P, :, :], GALL[P, :, co + pi * 2 : co + pi * 2 + 2], sgn,
                TGTC[P, :, csl], OP.mult, OP.subtract,
            )
            act.activation(AD[P, :, :], D[P, :, :], AF.Abs)
            vec.tensor_scalar(DM[P, :, :], AD[P, :, :], 1.0, None, OP.min)
            act.activation(
                junk16[P, 0 : NG * 2],
                DM[P, :, :].rearrange("p a b -> p (a b)"),
                AF.Square, scale=SQRT_HALF,
                accum_out=ACCB[P, 2 * pi : 2 * pi + 1],
            )
            vec.tensor_scalar(
                junk16[P, 0 : NG * 2],
                AD[P, :, :].rearrange("p a b -> p (a b)"), 1.0, -1.0,
                OP.max, OP.add,
                accum_out=ACCB[P, 2 * pi + 1 : 2 * pi + 2],
            )

    # cls CE: logsumexp(L) - L[y]
    for s in range(2):
        P = slice(64 * s, 64 * s + 64)
        co = NV * s + 4
        L = GALL[P, :, co : co + 4]
        E = small_p.tile([128, NG, C], F32, tag="E")
        SE = small_p.tile([128, NG], F32, tag="SE")
        act.activation(E[P, :, :], L, AF.Exp)
        vec.tensor_reduce(SE[P, :], E[P, :, :], AX.X, OP.add)
        act.activation(junk16[P, 0:NG], SE[P, :], AF.Ln,
                       accum_out=ACCC[P, 2 * s : 2 * s + 1])
        ZY = small_p.tile([128, NG, C], F32, tag="ZY")
        vec.tensor_tensor(ZY[P, :, :], L, Y[P, :, :], OP.mult)
        ZYS = small_p.tile([128, NG], F32, tag="ZYS")
        vec.tensor_reduce(ZYS[P, :], ZY[P, :, :], AX.X, OP.add)
        vec.tensor_scalar(junk16[P, 0:NG], ZYS[P, :], -1.0, 0.0,
                          OP.mult, OP.add,
                          accum_out=ACCC[P, 2 * s + 1 : 2 * s + 2])

    # conf: sum softplus(x) - sum x*pos (pos without multiplicity dedup)
    SPA = big_p.tile([128, 1], F32, tag="SPA")
    SPB = big_p.tile([128, 1], F32, tag="SPB")
    x4flat = X[:, :, :, 4].rearrange("p s j -> p (s j)")
    SA = big_p.tile([128, BC * NJ], F32, tag="SA")
    act.activation(SA[:, :], x4flat, AF.Abs)
    act.activation(SA[:, :], SA[:, :], AF.Exp, scale=-1.0)
    act.activation(junk16[:, :], SA[:, :], AF.Ln, bias=1.0,
                   accum_out=SPA[:, :])
    act.activation(junk16[:, :], x4flat, AF.Relu, accum_out=SPB[:, :])
    SP = big_p.tile([128, 1], F32, tag="SP")
    vec.tensor_tensor(SP[:, :], SPA[:, :], SPB[:, :], OP.add)

    # posdot: sum over g of gathered conf (col 8 for A half, 17 for B half)
    PR2 = big_p.tile([128, 2], F32, tag="PR2")
    vec.tensor_reduce(PR2[:, 0:1], GALL[:, :, 8], AX.X, OP.add)
    vec.tensor_reduce(PR2[:, 1:2], GALL[:, :, 17], AX.X, OP.add)

    # combine: OV cols = [box, cls, softplus, posdot]
    OV = big_p.tile([128, 4], F32, tag="OV")
    vec.tensor_reduce(OV[:, 0:1], ACCB[:, :], AX.X, OP.add)
    vec.tensor_reduce(OV[:, 1:2], ACCC[:, :], AX.X, OP.add)
    vec.tensor_copy(OV[:, 2:3], SP[:, :])
    PM = big_p.tile([128, 2], F32, tag="PM")
    vec.tensor_tensor(PM[:, :], PR2[:, :], MA[:, :], OP.mult)
    vec.tensor_reduce(OV[:, 3:4], PM[:, :], AX.X, OP.add)

    red_ps = psQ_p.tile([4, 1], F32, tag="red")
    nc.tensor.matmul(red_ps[:, :], OV[:, :], onescol[:, :], start=True, stop=True)
    outs = small_p.tile([4, 1], F32, tag="outs")
    vec.tensor_copy(outs[:, :], red_ps[:, :])
    nc.sync.dma_start(out_d[:].rearrange("(x o) -> x o", o=1), outs[:, :])


_NC = None
TRACE = False
LAST_RESULT = None


def _get_nc():
    global _NC
    if _NC is None:
        _NC = build_kernel()
    return _NC


def _csel():
    c = np.zeros((2, NG * 5, 128), dtype=np.float16)
    c[0, :, 0:64] = 1.0
    c[1, :, 64:128] = 1.0
    # q4 (sab) slots are negated so rs = 1/(-sab) < 0
    for g in range(NG):
        c[0, g * 5 + 4, 0:64] = -1.0
        c[1, g * 5 + 4, 64:128] = -1.0
    return c


def kernel(predictions, target_boxes, target_classes):
    nc = _get_nc()
    csel = _csel()
    in_maps = []
    for c in range(NCORES):
        sl = slice(c * BC, (c + 1) * BC)
        in_maps.append(
            {
                "predictions": np.ascontiguousarray(predictions[sl]),
                "target_boxes": np.ascontiguousarray(target_boxes[sl]),
                "target_classes": np.ascontiguousarray(target_classes[sl]),
                "csel": csel,
            }
        )
    global LAST_RESULT
    LAST_RESULT = run_bass_kernel_spmd(
        nc, in_maps, list(range(NCORES)), trace=TRACE
    )
    res = LAST_RESULT.results
    box = np.float64(0.0)
    cls_ = np.float64(0.0)
    conf = np.float64(0.0)
    for c in range(NCORES):
        o = np.asarray(res[c]["out"], dtype=np.float64)
        box += o[0]
        cls_ += o[1]
        conf += o[2] - o[3]
    total = (5.0 * box + 1.0 * cls_ + conf) / B
    return np.float32(total)


# revision 63
# speedup vs baseline: 1.1413x; 1.1413x over previous
"""DetectionLoss Trainium2 kernel (8-core data parallel), v3.

Per-core: 64 samples; groups of 2 samples -> 128 partitions = (s, t).
Free dim = 1280 preds (10 tiles of 128, 104 pad).

Score (order-equivalent to IoU): S = dx*dy/(ab+at) with
  dx = relu(WT - relu(bx1-tx1) - relu(tx2-bx2))        (exact)
5 PE streams per chunk, K=3 weights [selA, selB, target-row] so the
target offsets ride in the matmul; one 4-stream ACT relu evacuates
i1x,i1y,i2x,i2y from a 4-bank PSUM tile; the SAB stream is negated in
its weights so the final sign works out with eym<=0.

fp16 everywhere on SBUF (DVE 2x modes); fp32 only in PSUM + reciprocal.
No GPSIMD compute in the hot loop (Pool shares the DVE SBUF port and
poisons VEC throughput); Pool only issues DMAs / partition_broadcast.

Gather of matched box/logits/conf via exact index-compare one-hot
(transposed) matmuls; pos-mask term skips multiplicity dedup (error
~1.5e-4, measured).
"""

import numpy as np
import os
from contextlib import ExitStack

import concourse.bass as bass
import concourse.mybir as mybir
from concourse import bacc, tile
from concourse.bass_utils import run_bass_kernel_spmd

F32 = mybir.dt.float32
F16 = mybir.dt.float16
I32 = mybir.dt.int32
U32 = mybir.dt.uint32
OP = mybir.AluOpType
AF = mybir.ActivationFunctionType
AX = mybir.AxisListType

B, N, T, C = 512, 1176, 64, 4
NCORES = 8
BC = B // NCORES          # samples per core = 64
NG = BC // 2              # groups of 2 samples = 32
NJ = 10                   # pred tiles of 128 (padded)
NN = NJ * 128             # 1280
NTAIL = N - 9 * 128       # 24 valid rows in tile j=9
CHUNKS = [(0, 512), (512, 512), (1024, 256)]
IMG_W, IMG_H = 1472.0, 832.0
CS = 0.25                 # coordinate scale for fp16 range
LN4 = float(np.log(np.float64(4.0)))
SQRT_HALF = float(np.sqrt(np.float64(0.5)))
NV = 9                    # V cols per sample: 4 box, 4 logits, 1 conf
DBG = os.environ.get("KDEBUG", "") == "1"


def build_kernel():
    nc = bacc.Bacc(
        "TRN2",
        target_bir_lowering=False,
        debug=False,
        enable_asserts=False,
        num_devices=NCORES,
    )
    pred_d = nc.dram_tensor("predictions", [BC, N, 9], F32, kind="ExternalInput").ap()
    tb_d = nc.dram_tensor("target_boxes", [BC, T, 4], F32, kind="ExternalInput").ap()
    tc_d = nc.dram_tensor("target_classes", [BC, T], I32, kind="ExternalInput").ap()
    csel_d = nc.dram_tensor("csel", [2, NG * 5, 128], F16,
                            kind="ExternalInput").ap()
    out_d = nc.dram_tensor("out", [4], F32, kind="ExternalOutput").ap()
    dbg_d = (nc.dram_tensor("dbg", [128, NG], F32, kind="ExternalOutput").ap()
             if DBG else None)
    dbgs_d = (nc.dram_tensor("dbgS", [NG, 128, NN], F32, kind="ExternalOutput").ap()
              if DBG else None)
    dbgg_d = (nc.dram_tensor("dbgG", [128, NG, 32], F32, kind="ExternalOutput").ap()
              if DBG else None)

    with tile.TileContext(nc) as tcx:
        with ExitStack() as ctx:
            emit(ctx, tcx, pred_d, tb_d, tc_d, csel_d, out_d, dbg_d, dbgs_d,
                 dbgg_d)
    nc.compile()
    return nc


def emit(ctx, tcx, pred_d, tb_d, tc_d, csel_d, out_d, dbg_d=None, dbgs_d=None,
         dbgg_d=None):
    nc = tcx.nc
    tp = lambda name, bufs, **kw: ctx.enter_context(
        tcx.tile_pool(name=name, bufs=bufs, **kw)
    )

    const_p = tp("const", 1)
    big_p = tp("big", 1)
    work_p = tp("work", 2)
    wide_p = tp("wide", 1)
    sc_p = tp("sc", 2)
    small_p = tp("small", 3)
    stage_p = tp("stage", 2)
    psQ_p = tp("psQ", 1, space="PSUM")
    psB_p = tp("psB", 2, space="PSUM")
    psG_p = tp("psG", 1, space="PSUM")
    rs_p = tp("rs", 2)

    vec = nc.vector
    act = nc.scalar
    gps = nc.gpsimd

    # ---------------- constants ----------------
    onescol = const_p.tile([128, 1], F32, tag="onescol")
    vec.memset(onescol[:, :], 1.0)
    ONESROW = const_p.tile([1, 128], F16, tag="ONESROW")
    vec.memset(ONESROW[:, :], 1.0)
    # PJC[p, j, col] = p + 128*j  (fp16, exact up to 2048)
    pjc_i = const_p.tile([128, NJ, 128], mybir.dt.int16, tag="pjc_i")
    gps.iota(pjc_i[:, :, :], pattern=[[128, NJ], [0, 128]], base=0,
             channel_multiplier=1)
    PJC = const_p.tile([128, NJ, 128], F16, tag="PJC")
    vec.tensor_copy(PJC[:, :, :], pjc_i[:, :, :])
    # partition half masks
    MA = const_p.tile([128, 2], F32, tag="MA")
    vec.memset(MA[:, :], 0.0)
    vec.memset(MA[0:64, 0:1], 1.0)
    vec.memset(MA[64:128, 1:2], 1.0)

    # ---------------- loads ----------------
    # X[p, s, j, k] = pred[s, j*128+p, k]; pad rows zeroed, conf col -> -100
    X = big_p.tile([128, BC, NJ, 9], F32, tag="X")
    vec.memset(X[:, :, 9, :], 0.0)
    vec.memset(X[:, :, 9, 4], -100.0)
    nsplit = 2
    sw = BC // nsplit
    for i in range(nsplit):
        s0 = i * sw
        for j in range(NJ):
            pw = 128 if j < 9 else NTAIL
            eng = [nc.sync, nc.gpsimd, nc.scalar][(i * NJ + j) % 3]
            eng.dma_start(
                X[0:pw, s0 : s0 + sw, j, :],
                pred_d[s0 : s0 + sw, j * 128 : j * 128 + pw, :].rearrange(
                    "s p k -> p s k"
                ),
            )

    # targets: TGTC[p=(s,t), g, c] fp32
    TGTC = big_p.tile([128, NG, 4], F32, tag="TGTC")
    nc.sync.dma_start(TGTC[:, :, :], tb_d.rearrange("(g s) t c -> (s t) g c", s=2))
    TCI = big_p.tile([128, NG], I32, tag="TCI")
    nc.sync.dma_start(TCI[:, :], tc_d.rearrange("(g s) t -> (s t) g", s=2))
    TCF = big_p.tile([128, NG], F32, tag="TCF")
    vec.tensor_copy(TCF[:, :], TCI[:, :])

    # scaled target scalars (fp32):
    #  TGS cols: 0 tx1*cs, 1 ty1*cs, 2 tx2*cs, 3 ty2*cs, 4 wt*cs, 5 ht*cs,
    #            6 at*cs*cs
    TGS = big_p.tile([128, NG, 7], F32, tag="TGS")
    vec.tensor_scalar(TGS[:, :, 0:4], TGTC[:, :, :], CS, None, OP.mult)
    vec.tensor_tensor(TGS[:, :, 4], TGS[:, :, 2], TGS[:, :, 0], OP.subtract)
    vec.tensor_tensor(TGS[:, :, 5], TGS[:, :, 3], TGS[:, :, 1], OP.subtract)
    vec.tensor_tensor(TGS[:, :, 6], TGS[:, :, 4], TGS[:, :, 5], OP.mult)

    # ---------------- decode (fp32 in -> fp16 out) ----------------
    WHX = big_p.tile([128, BC, NJ], F16, tag="WHX")
    WHY = big_p.tile([128, BC, NJ], F16, tag="WHY")
    CX = big_p.tile([128, BC, NJ], F16, tag="CX")
    CY = big_p.tile([128, BC, NJ], F16, tag="CY")
    cb = const_p.tile([128, 3], F32, tag="cb")
    vec.memset(cb[:, 0:1], LN4)
    vec.memset(cb[:, 1:2], -IMG_W / 2 * CS)
    vec.memset(cb[:, 2:3], -IMG_H / 2 * CS)
    act.activation(WHX[:, :, :], X[:, :, :, 2], AF.Exp, bias=cb[:, 0:1], scale=1.0)
    act.activation(WHY[:, :, :], X[:, :, :, 3], AF.Exp, bias=cb[:, 0:1], scale=1.0)
    act.activation(CX[:, :, :], X[:, :, :, 0], AF.Identity, bias=cb[:, 1:2],
                   scale=IMG_W * CS)
    act.activation(CY[:, :, :], X[:, :, :, 1], AF.Identity, bias=cb[:, 2:3],
                   scale=IMG_H * CS)

    # DQ16[p, s, q*NJ+j], q: 0 bx1s, 1 by1s, 2 nbx2s, 3 nby2s, 4 abcc
    # (free dim padded to 128 for the XBAR DMA transpose)
    DQ16 = big_p.tile([128, BC, 128], F16, tag="DQ16")
    DQv = DQ16[:, :, 0 : 5 * NJ].rearrange("p s (q j) -> p s q j", q=5)
    vec.tensor_tensor(DQv[:, :, 0, :], CX[:, :, :], WHX[:, :, :], OP.subtract)
    vec.tensor_tensor(DQv[:, :, 1, :], CY[:, :, :], WHY[:, :, :], OP.subtract)
    vec.scalar_tensor_tensor(DQv[:, :, 2, :], CX[:, :, :], -1.0, WHX[:, :, :],
                             OP.mult, OP.subtract)
    vec.scalar_tensor_tensor(DQv[:, :, 3, :], CY[:, :, :], -1.0, WHY[:, :, :],
                             OP.mult, OP.subtract)
    vec.scalar_tensor_tensor(DQv[:, :, 4, :], WHX[:, :, :], 4.0, WHY[:, :, :],
                             OP.mult, OP.mult)

    # X16V[p, s, c, j]: c 0..3 = box (bx1s, by1s, nbx2s, nby2s), 4..7 logits,
    # 8 conf. fp16 gather source (lhsT of the gather matmuls).
    X16V = big_p.tile([128, BC, NV, NJ], F16, tag="X16V")
    vec.tensor_copy(
        X16V[:, :, 0:4, :],
        DQ16[:, :, 0 : 4 * NJ].rearrange("p s (c j) -> p s c j", c=4),
    )
    act.activation(
        X16V[:, :, 4:8, :],
        X[:, :, :, 5:9].rearrange("p s j c -> p s c j"),
        AF.Copy,
    )
    vec.tensor_copy(X16V[:, :, 8, :], X[:, :, :, 4])

    # one-hot class targets for CE
    Y = big_p.tile([128, NG, C], F32, tag="Y")
    for cc in range(C):
        vec.tensor_scalar(Y[:, :, cc], TCF[:, :], float(cc), None, OP.is_equal)

    # ---------------- K=3 stream weights ----------------
    # SEL5[:, f=g*5+q, :]: rows (selA, selB, target-row(q, g)).
    # q4 rows are NEGATED (so rs<0 and S = dx*(-eym)*(-rs) >= 0 works out).
    # target rows: q0 -tx1s, q1 -ty1s, q2 +tx2s, q3 +ty2s, q4 -at*cc.
    T5cc = stage_p.tile([128, 256], F16, tag="T5cc")
    vec.memset(T5cc[:, :], 0.0)
    vec.tensor_scalar(T5cc[:, 0 : 5 * NG : 5], TGS[:, :, 0], -1.0, None, OP.mult)
    vec.tensor_scalar(T5cc[:, 1 : 5 * NG : 5], TGS[:, :, 1], -1.0, None, OP.mult)
    vec.tensor_copy(T5cc[:, 2 : 5 * NG : 5], TGS[:, :, 2])
    vec.tensor_copy(T5cc[:, 3 : 5 * NG : 5], TGS[:, :, 3])
    vec.tensor_scalar(T5cc[:, 4 : 5 * NG : 5], TGS[:, :, 6], -1.0, None, OP.mult)
    T5a = stage_p.tile([128, 128], F16, tag="T5a")
    T5b = stage_p.tile([128, 128], F16, tag="T5b")
    nc.sync.dma_start_transpose(T5a[:, :], T5cc[:, 0:128])
    nc.sync.dma_start_transpose(T5b[:, :], T5cc[:, 128:256])
    SEL5 = const_p.tile([3, NG * 5, 128], F16, tag="SEL5")
    nc.sync.dma_start(SEL5[0:2, :, :], csel_d[:, :, :])
    nc.scalar.dma_start(SEL5[2:3, 0:128, :], T5a[:, :])
    nc.scalar.dma_start(SEL5[2:3, 128 : 5 * NG, :], T5b[0 : 5 * NG - 128, :])

    # persistent pred-row tiles (manually double-buffered; row2 = ones,
    # written once here, rows 0/1 rewritten per group by the reshape DMAs)
    PRS = [big_p.tile([3, 5, NN], F16, tag=f"PR{i}", name=f"PR{i}")
           for i in range(2)]
    for pr in PRS:
        gps.dma_start(
            pr[2:3, :, :].rearrange("one q (j p) -> one (q j) p", p=128),
            ONESROW[:, :].unsqueeze(1).broadcast_to((1, 5 * NJ, 128)),
        )

    # accumulators
    ACCB = big_p.tile([128, 8], F32, tag="ACCB")
    ACCC = big_p.tile([128, 4], F32, tag="ACCC")
    vec.memset(ACCB[:, :], 0.0)
    vec.memset(ACCC[:, :], 0.0)
    GALL = big_p.tile([128, NG, 32], F16, tag="GALL")
    junk16 = big_p.tile([128, BC * NJ], F16, tag="junk16")

    # ---------------- per-group software pipeline ----------------
    # iteration g: emit PE matmuls + ACT evacs for group g, interleaved with
    # the full post chain (scores/argmax/gather) for group g-1, so every
    # engine's in-order queue always has ready work.
    def prep(g):
        PR = PRS[g % 2]
        for s in range(2):
            Tst = stage_p.tile([128, 128], F16, tag="Tst", name="Tst")
            nc.sync.dma_start_transpose(Tst[:, :], DQ16[:, 2 * g + s, :])
            gps.dma_start(
                PR[s : s + 1, :, :].rearrange("one q (j p) -> one (q j) p",
                                              p=128),
                Tst[0 : 5 * NJ, :],
            )

    def mm_stage(g):
        PR = PRS[g % 2]
        R4 = work_p.tile([128, 4, NN], F16, tag="R4", name="R4")
        psBs = []
        for ci, (c0, cw) in enumerate(CHUNKS):
            psA = psQ_p.tile([128, 4, 512], F32, tag="psA", name="psA")
            psB = psB_p.tile([128, 512], F32, tag="psB", name="psB")
            for q in range(4):
                nc.tensor.matmul(
                    psA[:, q, 0:cw], SEL5[:, g * 5 + q, :],
                    PR[:, q, c0 : c0 + cw], start=True, stop=True,
                )
            nc.tensor.matmul(
                psB[:, 0:cw], SEL5[:, g * 5 + 4, :], PR[:, 4, c0 : c0 + cw],
                start=True, stop=True,
            )
            # one ACT op evacuates all 4 coordinate streams (relu, fp16)
            act.activation(R4[:, :, c0 : c0 + cw], psA[:, :, 0:cw], AF.Relu)
            psBs.append(psB)
        return R4, psBs

    def recip_chunk(st, ci):
        c0, cw = CHUNKS[ci]
        vec.reciprocal_approx_fast(st["RS32"][:, c0 : c0 + cw],
                                   st["psBs"][ci][:, 0:cw])

    def post_stage(g, st):
        R4 = st["R4"]
        RS32 = st["RS32"]
        SXY = wide_p.tile([128, 2, NN], F16, tag="SXY", name="SXY")
        vec.tensor_tensor(SXY[:, :, :], R4[:, 0:2, :], R4[:, 2:4, :], OP.add)
        DX = wide_p.tile([128, NN], F16, tag="DX", name="DX")
        act.activation(DX[:, :], SXY[:, 0, :], AF.Relu, bias=TGS[:, g, 4:5],
                       scale=-1.0)
        DY = wide_p.tile([128, NN], F16, tag="DY", name="DY")
        vec.tensor_scalar(DY[:, :], SXY[:, 1, :], TGS[:, g, 5:6], 0.0,
                          OP.subtract, OP.min)
        IPN = wide_p.tile([128, NN], F16, tag="IPN", name="IPN")
        vec.tensor_tensor(IPN[:, :], DX[:, :], DY[:, :], OP.mult)
        S = sc_p.tile([128, NN], F16, tag="S", name="S")
        vec.tensor_tensor(S[:, :], IPN[:, :], RS32[:, :], OP.mult)

        v8t = small_p.tile([128, 8], F16, tag="v8t", name="v8t")
        vec.max(v8t[:, :], S[:, :])
        idx8 = small_p.tile([128, 8], U32, tag="idx8", name="idx8")
        vec.max_index(idx8[:, :], v8t[:, :], S[:, :])
        mf16 = small_p.tile([128, 1], F16, tag="mf16", name="mf16")
        vec.tensor_copy(mf16[:, :], idx8[:, 0:1])
        if DBG:
            mf32 = small_p.tile([128, 1], F32, tag="mf32", name="mf32")
            vec.tensor_copy(mf32[:, :], idx8[:, 0:1])
            nc.sync.dma_start(dbg_d[:, g : g + 1], mf32[:, :])
            S32 = sc_p.tile([128, NN], F32, tag="S32", name="S32")
            vec.tensor_copy(S32[:, :], S[:, :])
            nc.sync.dma_start(dbgs_d[g, :, :], S32[:, :])

        MROW = small_p.tile([1, 128], F16, tag="MROW", name="MROW")
        gps.dma_start(
            MROW[:, :].rearrange("one (p x) -> one p x", x=1), mf16[:, :]
        )
        MB = small_p.tile([128, 128], F16, tag="MB", name="MB")
        gps.partition_broadcast(MB[:, :], MROW[:, :])
        st["MB"] = MB

    def oht_gather(g, st):
        OHT = sc_p.tile([128, NJ, 128], F16, tag="OHT", name="OHT")
        vec.tensor_tensor(
            OHT[:, :, :], PJC[:, :, :],
            st["MB"][:, :].unsqueeze(1).broadcast_to((128, NJ, 128)),
            OP.is_equal,
        )
        GT = psG_p.tile([2 * NV, 128], F32, tag="GT", name="GT")
        for j in range(NJ):
            nc.tensor.matmul(
                GT[:, :],
                X16V[:, 2 * g : 2 * g + 2, :, j].rearrange("p s c -> p (s c)"),
                OHT[:, j, :],
                start=(j == 0), stop=(j == NJ - 1),
            )
        GTS = stage_p.tile([32, 128], F16, tag="GTS", name="GTS")
        act.activation(GTS[0 : 2 * NV, :], GT[:, :], AF.Copy)
        nc.sync.dma_start_transpose(GALL[:, g, :], GTS[:, :])
        if DBG:
            G32 = small_p.tile([128, 32], F32, tag="G32", name="G32")
            vec.tensor_copy(G32[:, :], GALL[:, g, :])
            nc.sync.dma_start(dbgg_d[:, g, :], G32[:, :])

    prep(0)
    states = {}
    for g in range(NG + 1):
        if g < NG:
            if g + 1 < NG:
                prep(g + 1)
            R4, psBs = mm_stage(g)
            RS32 = rs_p.tile([128, NN], F32, tag="RS32", name="RS32")
            states[g] = {"R4": R4, "psBs": psBs, "RS32": RS32}
            recip_chunk(states[g], 0)
        if g >= 1:
            post_stage(g - 1, states[g - 1])
        if g < NG:
            recip_chunk(states[g], 1)
            recip_chunk(states[g], 2)
        if g >= 1:
            oht_gather(g - 1, states[g - 1])
            del states[g - 1]

    # ---------------- losses ----------------
    # box smooth-l1: d = |sgn*4*g - t|; sl1 = 0.5*min(d,1)^2 + max(d,1) - 1
    for s in range(2):
        P = slice(64 * s, 64 * s + 64)
        co = NV * s
        for pi, (csl, sgn) in enumerate([(slice(0, 2), 4.0), (slice(2, 4), -4.0)]):
            D = small_p.tile([128, NG, 2], F32, tag="D")
            AD = small_p.tile([128, NG, 2], F32, tag="AD")
            DM = small_p.tile([128, NG, 2], F16, tag="DM")
            vec.scalar_tensor_tensor(
                D[P, :, :], GALL[P, :, co + pi * 2 : co + pi * 2 + 2], sgn,
                TGTC[P, :, csl], OP.mult, OP.subtract,
            )
            act.activation(AD[P, :, :], D[P, :, :], AF.Abs)
            vec.tensor_scalar(DM[P, :, :], AD[P, :, :], 1.0, None, OP.min)
            act.activation(
                junk16[P, 0 : NG * 2],
                DM[P, :, :].rearrange("p a b -> p (a b)"),
                AF.Square, scale=SQRT_HALF,
                accum_out=ACCB[P, 2 * pi : 2 * pi + 1],
            )
            vec.tensor_scalar(
                junk16[P, 0 : NG * 2],
                AD[P, :, :].rearrange("p a b -> p (a b)"), 1.0, -1.0,
                OP.max, OP.add,
                accum_out=ACCB[P, 2 * pi + 1 : 2 * pi + 2],
            )

    # cls CE: logsumexp(L) - L[y]
    for s in range(2):
        P = slice(64 * s, 64 * s + 64)
        co = NV * s + 4
        L = GALL[P, :, co : co + 4]
        E = small_p.tile([128, NG, C], F32, tag="E")
        SE = small_p.tile([128, NG], F32, tag="SE")
        act.activation(E[P, :, :], L, AF.Exp)
        vec.tensor_reduce(SE[P, :], E[P, :, :], AX.X, OP.add)
        act.activation(junk16[P, 0:NG], SE[P, :], AF.Ln,
                       accum_out=ACCC[P, 2 * s : 2 * s + 1])
        ZY = small_p.tile([128, NG, C], F32, tag="ZY")
        vec.tensor_tensor(ZY[P, :, :], L, Y[P, :, :], OP.mult)
        ZYS = small_p.tile([128, NG], F32, tag="ZYS")
        vec.tensor_reduce(ZYS[P, :], ZY[P, :, :], AX.X, OP.add)
        vec.tensor_scalar(junk16[P, 0:NG], ZYS[P, :], -1.0, 0.0,
                          OP.mult, OP.add,
                          accum_out=ACCC[P, 2 * s + 1 : 2 * s + 2])

    # conf: sum softplus(x) - sum x*pos (pos without multiplicity dedup)
    SPA = big_p.tile([128, 1], F32, tag="SPA")
    SPB = big_p.tile([128, 1], F32, tag="SPB")
    x4flat = X[:, :, :, 4].rearrange("p s j -> p (s j)")
    SA = big_p.tile([128, BC * NJ], F32, tag="SA")
    act.activation(SA[:, :], x4flat, AF.Abs)
    act.activation(SA[:, :], SA[:, :], AF.Exp, scale=-1.0)
    act.activation(junk16[:, :], SA[:, :], AF.Ln, bias=1.0,
                   accum_out=SPA[:, :])
    act.activation(junk16[:, :], x4flat, AF.Relu, accum_out=SPB[:, :])
    SP = big_p.tile([128, 1], F32, tag="SP")
    vec.tensor_tensor(SP[:, :], SPA[:, :], SPB[:, :], OP.add)

    # posdot: sum over g of gathered conf (col 8 for A half, 17 for B half)
    PR2 = big_p.tile([128, 2], F32, tag="PR2")
    vec.tensor_reduce(PR2[:, 0:1], GALL[:, :, 8], AX.X, OP.add)
    vec.tensor_reduce(PR2[:, 1:2], GALL[:, :, 17], AX.X, OP.add)

    # combine: OV cols = [box, cls, softplus, posdot]
    OV = big_p.tile([128, 4], F32, tag="OV")
    vec.tensor_reduce(OV[:, 0:1], ACCB[:, :], AX.X, OP.add)
    vec.tensor_reduce(OV[:, 1:2], ACCC[:, :], AX.X, OP.add)
    vec.tensor_copy(OV[:, 2:3], SP[:, :])
    PM = big_p.tile([128, 2], F32, tag="PM")
    vec.tensor_tensor(PM[:, :], PR2[:, :], MA[:, :], OP.mult)
    vec.tensor_reduce(OV[:, 3:4], PM[:, :], AX.X, OP.add)

    red_ps = psQ_p.tile([4, 1], F32, tag="red")
    nc.tensor.matmul(red_ps[:, :], OV[:, :], onescol[:, :], start=True, stop=True)
    outs = small_p.tile([4, 1], F32, tag="outs")
    vec.tensor_copy(outs[:, :], red_ps[:, :])
    nc.sync.dma_start(out_d[:].rearrange("(x o) -> x o", o=1), outs[:, :])


_NC = None
TRACE = False
LAST_RESULT = None


def _get_nc():
    global _NC
    if _NC is None:
        _NC = build_kernel()
    return _NC


def _csel():
    c = np.zeros((2, NG * 5, 128), dtype=np.float16)
    c[0, :, 0:64] = 1.0
    c[1, :, 64:128] = 1.0
    # q4 (sab) slots are negated so rs = 1/(-sab) < 0
    for g in range(NG):
        c[0, g * 5 + 4, 0:64] = -1.0
        c[1, g * 5 + 4, 64:128] = -1.0
    return c


def kernel(predictions, target_boxes, target_classes):
    nc = _get_nc()
    csel = _csel()
    in_maps = []
    for c in range(NCORES):
        sl = slice(c * BC, (c + 1) * BC)
        in_maps.append(
            {
                "predictions": np.ascontiguousarray(predictions[sl]),
                "target_boxes": np.ascontiguousarray(target_boxes[sl]),
                "target_classes": np.ascontiguousarray(target_classes[sl]),
                "csel": csel,
            }
        )
    global LAST_RESULT
    LAST_RESULT = run_bass_kernel_spmd(
        nc, in_maps, list(range(NCORES)), trace=TRACE
    )
    res = LAST_RESULT.results
    box = np.float64(0.0)
    cls_ = np.float64(0.0)
    conf = np.float64(0.0)
    for c in range(NCORES):
        o = np.asarray(res[c]["out"], dtype=np.float64)
        box += o[0]
        cls_ += o[1]
        conf += o[2] - o[3]
    total = (5.0 * box + 1.0 * cls_ + conf) / B
    return np.float32(total)
